# revision 14
# baseline (speedup 1.0000x reference)
"""Cepstrum -> minimum-phase impulse response on 8 Trainium2 NeuronCores.

Math: the reference recurrence  n*h_n = sum_k (k c_k) h_{n-k}, h_0 = exp(c_0)
is exactly the power-series exponential h = exp(C(z)) mod z^512 for the
degree-255 polynomial C. We evaluate it spectrally with an L=512 DFT:

    h = IDFT_512( exp( DFT_512(c) ) )

exact up to aliasing of exp(C)'s tail beyond degree 511 (~1e-4 abs; C^2
reaches only degree 510, so only C^3/6-and-up alias back).

The inverse transform is split radix-2 by bin parity (h real, Hermitian H):
  even bins k=2j   -> P_n, period 256:        P_{n+256} = +P_n
  odd  bins k=2j+1 -> Q_n, anti-period 256:   Q_{n+256} = -Q_n
and each branch folds again via n <-> 256-n mirror symmetry:
  P_n = Ee_n + Oe_n,  P_{256-n} = Ee_n - Oe_n     (n = 0..128)
  Q_n = Eo_n + Oo_n,  Q_{256-n} = Oo_n - Eo_n
so the inverse matmuls produce only 4 x 129 columns per row (4x fewer MACs
than a flat 512-column inverse). The device stores U=Ee+Oe, V=Ee-Oe,
W=Eo+Oo, Z=Oo-Eo; the final radix-2 butterfly h = P +- Q (0.05% of FLOPs)
happens during host-side gather.

Device pipeline per 512-row supertile (all matmuls fp16 in / fp32 psum):
  - fwd:  R/I[bins, rows] = W-stationary matmuls over cT, branch bc=0: even
          bins 2..256, bc=1: odd bins 1..255
  - ACT:  A = exp(R), Sn = sin2pi(I/2pi), Cs = sin2pi(I/2pi + 1/4), fp16 out
          (single ACT table set: exp_and_friends = {exp, sin2pi})
  - DVE:  HRe = A*Cs, HIm = A*Sn  (fp16, 2x packed mode)
  - inv:  per branch one PSUM tile [128, 258] = [E | O], 1 matmul each
  - ACT:  stage O-halves to SBUF (DVE may read only one PSUM operand)
  - DVE:  U/V/W/Z = E +- O into one fp16 tile, DMA out.

Host-side (not device work): shard rows across 8 cores, pre-transpose c to
(256, rows) fp16; during gather apply the butterfly h_n = P_n + Q_n,
h_{n+256} = P_n - Q_n, add the DC term exp(sum c)/L and the fp16-weight
rounding compensation rows corrU/corrV.
"""

import os
import sys
from contextlib import ExitStack

import numpy as np

for _p in ("/opt/trn_rl_repo", "/root/.axon_site/_ro/trn_rl_repo"):
    if os.path.isdir(_p) and _p not in sys.path:
        sys.path.insert(0, _p)

from concourse import bacc, mybir, tile  # noqa: E402
from concourse.bass_utils import run_bass_kernel_spmd  # noqa: E402

B_TOTAL = 131072
N_CORES = 8
B_CORE = B_TOTAL // N_CORES  # 16384
M1 = 256          # cepstral coefficients per row (M+1)
N_OUT = 512       # impulse response length
L = 512           # DFT length (aliasing ~1e-4 abs, validated offline)
NH = 128          # bins per parity branch
NF = NH + 1       # 129 folded inverse columns per branch
ST_ROWS = 512     # rows per supertile
N_ST = B_CORE // ST_ROWS  # 32

F32 = mybir.dt.float32
F16 = mybir.dt.float16

_cache: dict = {}

TWO_PI = 2.0 * np.pi


def _install_sin2pi_patches():
    """Keep all activations in ONE ACT table set (exp_and_friends = {exp,
    sin2pi}) to avoid per-supertile table reloads (~2.7us each).

    1. Patch bacc's activation-table map so Exp and Sin both resolve to
       exp_and_friends -> bacc emits a single LoadActFuncSet.
    2. Rewrite "Sin" -> "Sin2pi" in the BIR json just before walrus; the
       kernel emits Sin with scale=1/(2*pi) so the arguments are already
       in sin2pi's convention (sin2pi(x) = sin(2*pi*x)).
    """
    if _cache.get("patched"):
        return
    import concourse.bacc as _bacc
    import concourse.bass2jax as _b2j

    SIN = mybir.ActivationFunctionType.Sin
    EXP = mybir.ActivationFunctionType.Exp
    _orig_tables = _bacc.get_activation_tables

    def tables_patched(arch):
        t = {k: set(v) for k, v in _orig_tables(arch).items()}
        for k in t:
            t[k].discard(SIN)
            if k != "exp_and_friends":
                t[k].discard(EXP)
        if "exp_and_friends" in t:
            t["exp_and_friends"] |= {SIN, EXP}
        return t

    _bacc.get_activation_tables = tables_patched

    _orig_compile = _b2j.compile_bir_kernel

    def compile_patched(bir_json, *a, **kw):
        # only rewrite THIS kernel's module (identified by its weight tensor)
        if isinstance(bir_json, bytes):
            if b'"wcf"' in bir_json:
                bir_json = bir_json.replace(b'"func":"Sin"', b'"func":"Sin2pi"')
        elif '"wcf"' in bir_json:
            bir_json = bir_json.replace('"func":"Sin"', '"func":"Sin2pi"')
        return _orig_compile(bir_json, *a, **kw)

    _b2j.compile_bir_kernel = compile_patched
    _cache["patched"] = True


def _host_weights():
    d = np.arange(M1, dtype=np.float64)
    ke = 2.0 * np.arange(1, NH + 1, dtype=np.float64)      # even bins 2..256
    ko = 2.0 * np.arange(0, NH, dtype=np.float64) + 1.0    # odd bins 1..255
    kk = np.concatenate([ke, ko])                          # fwd col order
    th = 2.0 * np.pi * np.outer(d, kk) / L                 # (256, 256)
    wc = np.cos(th)
    ws = -np.sin(th)

    n = np.arange(NF, dtype=np.float64)
    we = np.where(ke == L // 2, 1.0, 2.0)[:, None] / L     # Nyquist: 1/L
    wo = np.full((NH, 1), 2.0) / L
    thiE = 2.0 * np.pi * np.outer(ke, n) / L               # (128, 129)
    thiO = 2.0 * np.pi * np.outer(ko, n) / L
    ciE = we * np.cos(thiE)
    ciO = wo * np.cos(thiO)
    ci = np.concatenate([ciE, ciO], axis=0)                # (256, 129)
    si = np.concatenate([-we * np.sin(thiE), -wo * np.sin(thiO)], axis=0)
    ci16 = ci.astype(np.float16)
    si16 = si.astype(np.float16)
    # Coherent fp16 rounding of the ci columns (H ~= 1 background does not
    # cancel it): add back on the host, per butterfly sign.
    dE = (ci16[:NH].astype(np.float64) - ciE).sum(0)       # (129,)
    dO = (ci16[NH:].astype(np.float64) - ciO).sum(0)
    corrU = (-dE - dO).astype(np.float32)
    corrV = (-dE + dO).astype(np.float32)
    return wc.astype(np.float16), ws.astype(np.float16), ci16, si16, corrU, corrV


def _build(n_st=N_ST, repeat=1, ob=16, ab=8, hb=8, cb=6, rib=2, eob=2):
    _install_sin2pi_patches()
    nc = bacc.Bacc(
        "TRN2", target_bir_lowering=False, debug=False, num_devices=N_CORES
    )
    ct_ap = nc.dram_tensor("ct", [M1, n_st * ST_ROWS], F16, kind="ExternalInput").ap()
    wc_ap = nc.dram_tensor("wcf", [M1, 2 * NH], F16, kind="ExternalInput").ap()
    ws_ap = nc.dram_tensor("wsf", [M1, 2 * NH], F16, kind="ExternalInput").ap()
    ci_ap = nc.dram_tensor("cif", [2 * NH, NF], F16, kind="ExternalInput").ap()
    si_ap = nc.dram_tensor("sif", [2 * NH, NF], F16, kind="ExternalInput").ap()
    h_ap = nc.dram_tensor("h", [n_st * ST_ROWS, N_OUT], F16, kind="ExternalOutput").ap()

    EXP = mybir.ActivationFunctionType.Exp
    SIN = mybir.ActivationFunctionType.Sin

    with tile.TileContext(nc) as tc, ExitStack() as ctx:
        const = ctx.enter_context(tc.tile_pool(name="const", bufs=1))
        ctp = ctx.enter_context(tc.tile_pool(name="ctp", bufs=cb))
        actp = ctx.enter_context(tc.tile_pool(name="actp", bufs=ab))
        hp = ctx.enter_context(tc.tile_pool(name="hp", bufs=hb))
        outp = ctx.enter_context(tc.tile_pool(name="outp", bufs=ob))
        ps_ri = ctx.enter_context(tc.tile_pool(name="ps_ri", bufs=rib, space="PSUM"))
        ps_eo = ctx.enter_context(tc.tile_pool(name="ps_eo", bufs=eob, space="PSUM"))

        # constants
        wc_sb = [const.tile([128, 2 * NH], F16, tag=f"wc{d}", name=f"wc{d}") for d in range(2)]
        ws_sb = [const.tile([128, 2 * NH], F16, tag=f"ws{d}", name=f"ws{d}") for d in range(2)]
        for d in range(2):
            nc.sync.dma_start(wc_sb[d][:], wc_ap[d * 128:(d + 1) * 128, :])
            nc.sync.dma_start(ws_sb[d][:], ws_ap[d * 128:(d + 1) * 128, :])
        ci_sb = [const.tile([128, NF], F16, tag=f"ci{b}", name=f"ci{b}") for b in range(2)]
        si_sb = [const.tile([128, NF], F16, tag=f"si{b}", name=f"si{b}") for b in range(2)]
        for b in range(2):
            nc.sync.dma_start(ci_sb[b][:], ci_ap[b * 128:(b + 1) * 128, :])
            nc.sync.dma_start(si_sb[b][:], si_ap[b * 128:(b + 1) * 128, :])
        zb = const.tile([128, 1], F32, tag="zb")
        nc.gpsimd.memset(zb[:], 0.0)
        quarter = const.tile([128, 1], F32, tag="quarter")
        nc.gpsimd.memset(quarter[:], 0.25)

        for st in range(n_st * repeat):
            st = st % n_st
            r0 = st * ST_ROWS
            # ---- load pre-transposed cT chunks ----
            cT16 = []
            for d in range(2):
                t = ctp.tile([128, ST_ROWS], F16, tag="cT16", name="cT16")
                nc.sync.dma_start(t[:], ct_ap[d * 128:(d + 1) * 128, r0:r0 + ST_ROWS])
                cT16.append(t)

            # ---- forward DFT + pointwise, per parity branch ----
            HRe = []
            HIm = []
            for bc in range(2):
                r_ps = ps_ri.tile([128, ST_ROWS], F32, tag="R")
                i_ps = ps_ri.tile([128, ST_ROWS], F32, tag="I")
                for d in range(2):
                    nc.tensor.matmul(
                        r_ps[:], wc_sb[d][:, bc * 128:(bc + 1) * 128], cT16[d][:],
                        start=(d == 0), stop=(d == 1),
                    )
                for d in range(2):
                    nc.tensor.matmul(
                        i_ps[:], ws_sb[d][:, bc * 128:(bc + 1) * 128], cT16[d][:],
                        start=(d == 0), stop=(d == 1),
                    )
                a_sb = actp.tile([128, ST_ROWS], F16, tag="A")
                sn_sb = actp.tile([128, ST_ROWS], F16, tag="Sn")
                cs_sb = actp.tile([128, ST_ROWS], F16, tag="Cs")
                nc.scalar.activation(a_sb[:], r_ps[:], EXP, bias=zb[:])
                nc.scalar.activation(sn_sb[:], i_ps[:], SIN, bias=zb[:],
                                     scale=float(1.0 / TWO_PI))
                nc.scalar.activation(cs_sb[:], i_ps[:], SIN, bias=quarter[:],
                                     scale=float(1.0 / TWO_PI))
                hre = hp.tile([128, ST_ROWS], F16, tag="HRe")
                him = hp.tile([128, ST_ROWS], F16, tag="HIm")
                nc.vector.tensor_mul(hre[:], a_sb[:], cs_sb[:])
                nc.vector.tensor_mul(him[:], a_sb[:], sn_sb[:])
                HRe.append(hre)
                HIm.append(him)

            # ---- double-folded inverse per row-chunk + store ----
            # Per branch one PSUM bank [128, 258] = [E | O]; DVE may read
            # only one PSUM operand, so stage the O halves via ACT copies.
            for rc in range(4):
                rs = slice(rc * 128, (rc + 1) * 128)
                eo0 = ps_eo.tile([128, 2 * NF], F32, tag="eo0")
                eo1 = ps_eo.tile([128, 2 * NF], F32, tag="eo1")
                nc.tensor.matmul(eo0[:, 0:NF], HRe[0][:, rs], ci_sb[0][:],
                                 start=True, stop=True)
                nc.tensor.matmul(eo0[:, NF:2 * NF], HIm[0][:, rs], si_sb[0][:],
                                 start=True, stop=True)
                nc.tensor.matmul(eo1[:, 0:NF], HRe[1][:, rs], ci_sb[1][:],
                                 start=True, stop=True)
                nc.tensor.matmul(eo1[:, NF:2 * NF], HIm[1][:, rs], si_sb[1][:],
                                 start=True, stop=True)
                oe_st = outp.tile([128, NF], F16, tag="oe_st")
                oo_st = outp.tile([128, NF], F16, tag="oo_st")
                nc.scalar.copy(oe_st[:], eo0[:, NF:2 * NF])
                nc.scalar.copy(oo_st[:], eo1[:, NF:2 * NF])
                o_sb = outp.tile([128, N_OUT], F16, tag="o_sb")
                # U | V | W | Z  (129 | 127 | 129 | 127 columns)
                nc.vector.tensor_add(o_sb[:, 0:NF], eo0[:, 0:NF], oe_st[:])
                nc.vector.tensor_sub(o_sb[:, NF:256], eo0[:, 1:NH], oe_st[:, 1:NH])
                nc.vector.tensor_add(o_sb[:, 256:256 + NF], eo1[:, 0:NF], oo_st[:])
                nc.vector.tensor_sub(o_sb[:, 256 + NF:N_OUT], oo_st[:, 1:NH],
                                     eo1[:, 1:NH])
                nc.sync.dma_start(h_ap[r0 + rc * 128: r0 + (rc + 1) * 128, :],
                                  o_sb[:])

    nc.compile()
    return nc


def _get_nc(n_st=N_ST):
    key = ("nc", n_st)
    if key not in _cache:
        _cache[key] = _build(n_st)
    return _cache[key]


def _in_maps(c):
    wc, ws, ci, si, _, _ = _host_weights()
    maps = []
    for i in range(N_CORES):
        ct = np.ascontiguousarray(
            c[i * B_CORE:(i + 1) * B_CORE].astype(np.float16).T
        )
        maps.append({"ct": ct, "wcf": wc, "wsf": ws, "cif": ci, "sif": si})
    return maps


def _assemble(h16, c):
    """Host-side gather: radix-2 butterfly h_n = P_n + Q_n,
    h_{n+256} = P_n - Q_n, plus DC bin and fp16-weight compensation."""
    _, _, _, _, corrU, corrV = _host_weights()
    dc = (np.exp(c.astype(np.float64).sum(axis=1)) / L).astype(np.float32)
    U = h16[:, 0:NF].astype(np.float32)           # P_n,      n = 0..128
    V = h16[:, NF:256].astype(np.float32)         # P_{256-n}, n = 1..127
    W = h16[:, 256:256 + NF].astype(np.float32)   # Q_n,      n = 0..128
    Z = h16[:, 256 + NF:].astype(np.float32)      # Q_{256-n}, n = 1..127
    out = np.empty((c.shape[0], N_OUT), np.float32)
    b = dc[:, None]
    out[:, 0:NF] = U + W + (b + corrU[None, :])
    out[:, NF:256] = (V + Z)[:, ::-1] + (b + corrV[None, NH - 1:0:-1])
    out[:, 256:256 + NF] = U - W + (b + corrV[None, :])
    out[:, 256 + NF:] = (V - Z)[:, ::-1] + (b + corrU[None, NH - 1:0:-1])
    return out


def kernel(c):
    c = np.ascontiguousarray(np.asarray(c), dtype=np.float32)
    assert c.shape == (B_TOTAL, M1), c.shape
    nc = _get_nc()
    res = run_bass_kernel_spmd(nc, _in_maps(c), list(range(N_CORES)))
    h16 = np.concatenate(
        [res.results[i]["h"] for i in range(N_CORES)], axis=0
    )
    return _assemble(h16, c)
